# revision 14
# baseline (speedup 1.0000x reference)
"""Trainium2 Bass kernel for nn_CriticNetwork (GCN critic head), 8 cores.

Math (reference): h = GCNConv(x, edge_index); sv = relu(h[agent_idx]);
sv = relu(LN(sv@W1+b1)); sv = LN(sv@W2+b2); q = relu(sv + action@Wa+ba) @ Wq + bq.

Exact algebraic restructurings (no approximation):
  * GCNConv is linear-then-propagate, so aggregate in the 128-d INPUT space
    and apply Wg after:  z[v] = sum_{e:dst=v} norm_e * x[src_e],
    h[v] = z[v] @ Wg + bg, with norm_e = dinv[src]*dinv[dst] and the self
    loop as one more edge (v->v, norm dinv[v]^2).  Only agent rows are ever
    used downstream, so only edges landing on agent nodes are aggregated
    (~121k of 800k).
  * Per-edge norm scaling + segment-sum fuse into one PE matmul per
    128-slot tile:  zT += G_t^T @ S_t  where G_t = gathered x rows
    [slot, feat] and S_t[slot, agent] = norm (0 off-target).  The output is
    directly transposed ([feat, agent]), which the whole MLP consumes.
  * ba folds into be2 (both add before the final relu).

vs. the first working version: the per-tile indirect DMAs (994ns fixed
SWDGE cost x 152) are replaced by 8 large dma_gather instructions (two per
2-chunk pair; x is split at row 32768 so gather indices fit int16), and all
matmul operands are bf16 (fp32 PSUM accumulation), halving PE stream time
and all HBM traffic.  MLP block b runs as soon as its 4 aggregation chunks
are done, overlapping with the remaining gathers.

Sharding: agents split 1024/core (data parallel); host work is graph
preprocessing only (CSR bucketing, degree/norm coefficients, index tables,
sparse S blocks) -- all feature-tensor FLOPs run on device.
"""

import numpy as np
import ml_dtypes

import concourse.bass as bass
import concourse.mybir as mybir
import concourse.tile as tile
from concourse import library_config
from concourse.bass_utils import run_bass_kernel_spmd

N_NODES = 50000
D_IN = 128
D_HID = 256
FC1 = 512
FC2 = 256
N_ACT = 64
N_AGENTS = 8192
LN_EPS = 1e-5

N_CORES = 8
A_PER_CORE = N_AGENTS // N_CORES        # 1024
ABLK = 512                              # agent block width for MLP
N_ABLK = A_PER_CORE // ABLK             # 2
AGG_CHUNK = 128                         # agents per aggregation chunk
N_CHUNKS = A_PER_CORE // AGG_CHUNK      # 8

SPLIT = 32768                           # x row split for int16 gather idx
T_LO = 13                               # slot tiles per chunk from x[:SPLIT]
T_HI = 7                                # slot tiles per chunk from x[SPLIT:]
CT = T_LO + T_HI                        # 20 tiles per chunk buffer
N_LO = T_LO * 128                       # 1664 lo idxs per chunk
N_HI = T_HI * 128                       # 896 hi idxs per chunk
IDXW = (N_LO + N_HI) // 16              # 160 idx columns per chunk
# dma_gather tops out at 1024 idxs per instruction (desc-ring capacity):
# split each chunk's lo stream into 1024 + 640.
GATHERS = (                             # (src, idx_col0, n_idx, tile0)
    ('lo', 0, 1024, 0),
    ('lo', 64, N_LO - 1024, 8),
    ('hi', N_LO // 16, N_HI, T_LO),
)

FLOAT = mybir.dt.float32
BF16 = mybir.dt.bfloat16
AF = mybir.ActivationFunctionType


def _split_multi_waits(nc, max_waits=1):
    """This container's walrus rejects >1 sync-wait per instruction; move
    extras onto same-engine NoOps inserted right before (equivalent)."""
    for func in nc.m.functions:
        for bb in func.blocks:
            out, changed = [], False
            for inst in bb.instructions:
                si = inst.sync_info
                if si is not None and len(si.on_wait) > max_waits:
                    waits = list(si.on_wait)
                    extra, keep = waits[:-max_waits], waits[-max_waits:]
                    for k in range(0, len(extra), max_waits):
                        nop = mybir.InstNoOp(
                            name=nc.get_next_instruction_name(),
                            engine=inst.engine, bass_nofuse=True,
                            sync_info=mybir.SyncInfo(
                                on_wait=list(extra[k:k + max_waits]),
                                on_update=[]))
                        nc.register_instruction(nop)
                        out.append(nop)
                        changed = True
                    si.on_wait.clear()
                    si.on_wait.extend(keep)
                    inst.sync_info = si
                out.append(inst)
            if changed:
                bb.instructions = out


def _build_program():
    nc = bass.Bass(target_bir_lowering=False)

    xlo_t = nc.declare_dram_parameter('xlo', [SPLIT, D_IN], BF16, isOutput=False)
    xhi_t = nc.declare_dram_parameter(
        'xhi', [N_NODES - SPLIT, D_IN], BF16, isOutput=False)
    idx_t = nc.declare_dram_parameter(
        'sidx', [128, N_CHUNKS * IDXW], mybir.dt.int16, isOutput=False)
    s_t = nc.declare_dram_parameter(
        'smat', [N_CHUNKS, 128, CT * AGG_CHUNK], BF16, isOutput=False)
    act_t = nc.declare_dram_parameter(
        'actT', [N_ACT, A_PER_CORE], BF16, isOutput=False)
    wg_t = nc.declare_dram_parameter('Wg', [D_IN, D_HID], BF16, isOutput=False)
    w1_t = nc.declare_dram_parameter('W1s', [128, 2 * FC1], BF16, isOutput=False)
    w2_t = nc.declare_dram_parameter('W2s', [128, 4 * FC2], BF16, isOutput=False)
    wa_t = nc.declare_dram_parameter('Wa', [N_ACT, FC2], BF16, isOutput=False)
    wq_t = nc.declare_dram_parameter('Wqs', [128, 2], BF16, isOutput=False)
    bias_t = nc.declare_dram_parameter('biases', [7, FC1], FLOAT, isOutput=False)
    q_out = nc.declare_dram_parameter('q', [1, A_PER_CORE], FLOAT, isOutput=True)

    with tile.TileContext(nc) as tc:
        with (
            tc.tile_pool(name='const', bufs=1) as constp,
            tc.tile_pool(name='gath', bufs=3) as gathp,
            tc.tile_pool(name='smatp', bufs=3) as smatp,
            tc.tile_pool(name='zt', bufs=1) as ztp,
            tc.tile_pool(name='ps_z', bufs=2, space='PSUM') as ps_z,
            tc.tile_pool(name='ps_y', bufs=2, space='PSUM') as ps_y,
            tc.tile_pool(name='ps_x', bufs=1, space='PSUM') as ps_x,
            tc.tile_pool(name='ps_st', bufs=1, space='PSUM') as ps_st,
            tc.tile_pool(name='mlp', bufs=2) as mlp,
            tc.tile_pool(name='mlp4', bufs=4) as mlp4,
            tc.tile_pool(name='keep', bufs=1) as keep,
        ):
            nc.gpsimd.load_library(library_config.mlp)

            # ---------------- constants ----------------
            idx_sb = constp.tile([128, N_CHUNKS * IDXW], mybir.dt.int16)
            nc.sync.dma_start(out=idx_sb[:], in_=idx_t[:])
            wg = constp.tile([D_IN, D_HID], BF16)
            nc.sync.dma_start(out=wg[:], in_=wg_t[:])
            w1 = constp.tile([128, 2 * FC1], BF16)
            nc.sync.dma_start(out=w1[:], in_=w1_t[:])
            w2 = constp.tile([128, 4 * FC2], BF16)
            nc.sync.dma_start(out=w2[:], in_=w2_t[:])
            wa = constp.tile([N_ACT, FC2], BF16)
            nc.sync.dma_start(out=wa[:], in_=wa_t[:])
            wq = constp.tile([128, 2], BF16)
            nc.sync.dma_start(out=wq[:], in_=wq_t[:])
            actT = constp.tile([N_ACT, A_PER_CORE], BF16)
            nc.sync.dma_start(out=actT[:], in_=act_t[:])
            ones1 = constp.tile([128, 128], BF16)
            nc.vector.memset(ones1[:], 1.0 / FC1)
            ones2 = constp.tile([128, 128], BF16)
            nc.vector.memset(ones2[:], 1.0 / FC2)
            zero_col = constp.tile([128, 1], FLOAT)
            nc.vector.memset(zero_col[:], 0.0)
            eps_col = constp.tile([128, 1], FLOAT)
            nc.vector.memset(eps_col[:], LN_EPS)

            def bias_col(row, n):
                t = constp.tile([128, n // 128], FLOAT, tag=f'bias{row}')
                nc.sync.dma_start(
                    out=t[:],
                    in_=bias_t[row, 0:n].rearrange('(k p) -> p k', p=128))
                return t

            bgT = bias_col(0, D_HID)
            b1T = bias_col(1, FC1)
            g1T = bias_col(2, FC1)
            be1T = bias_col(3, FC1)
            b2T = bias_col(4, FC2)
            g2T = bias_col(5, FC2)
            be2T = bias_col(6, FC2)   # includes +ba
            bq_sb = constp.tile([1, 1], FLOAT)
            nc.sync.dma_start(out=bq_sb[:], in_=bias_t[4:5, 256:257])

            zt_sb = ztp.tile([D_IN, A_PER_CORE], BF16)

            # ------------- phase 2: MLP (transposed activations) -------------
            def ln_block(in_tiles, w, nin, nout, bT, gT, beT, ones, relu):
                """y = w^T in + b; LN over the nout*128 feature axis
                (= partition axis across tiles); optional relu.  Returns
                SBUF tiles [128, ABLK] * nout."""
                y_sb = []
                for o in range(nout):
                    ps = ps_y.tile([128, ABLK], FLOAT, tag='y')
                    for k in range(nin):
                        nc.tensor.matmul(
                            out=ps[:],
                            lhsT=w[:, (k * nout + o) * 128:(k * nout + o + 1) * 128],
                            rhs=in_tiles[k][:],
                            start=(k == 0), stop=(k == nin - 1))
                    sb = mlp4.tile([128, ABLK], BF16, tag='ysb')
                    nc.scalar.activation(out=sb[:], in_=ps[:], func=AF.Identity,
                                         bias=bT[:, o:o + 1], scale=1.0)
                    y_sb.append(sb)
                mu = ps_st.tile([128, ABLK], FLOAT, tag='stat')
                for o in range(nout):
                    nc.tensor.matmul(out=mu[:], lhsT=ones[:], rhs=y_sb[o][:],
                                     start=(o == 0), stop=(o == nout - 1))
                d_sb, sq_sb = [], []
                for o in range(nout):
                    d = mlp4.tile([128, ABLK], BF16, tag='d')
                    nc.vector.tensor_sub(out=d[:], in0=y_sb[o][:], in1=mu[:])
                    d_sb.append(d)
                    s = mlp4.tile([128, ABLK], BF16, tag='sq')
                    nc.scalar.activation(out=s[:], in_=d[:], func=AF.Square,
                                         bias=zero_col[:, 0:1])
                    sq_sb.append(s)
                var = ps_st.tile([128, ABLK], FLOAT, tag='stat')
                for o in range(nout):
                    nc.tensor.matmul(out=var[:], lhsT=ones[:], rhs=sq_sb[o][:],
                                     start=(o == 0), stop=(o == nout - 1))
                lg = mlp.tile([128, ABLK], FLOAT, tag='lg')
                nc.scalar.activation(out=lg[:], in_=var[:], func=AF.Ln,
                                     bias=eps_col[:, 0:1])
                r = mlp.tile([128, ABLK], BF16, tag='r')
                nc.scalar.activation(out=r[:], in_=lg[:], func=AF.Exp,
                                     bias=zero_col[:, 0:1], scale=-0.5)
                outs = []
                for o in range(nout):
                    t1 = mlp.tile([128, ABLK], BF16, tag='t1')
                    nc.vector.tensor_mul(out=t1[:], in0=d_sb[o][:], in1=r[:])
                    t3 = mlp4.tile([128, ABLK], BF16, tag='t3')
                    nc.scalar.activation(
                        out=t3[:], in_=t1[:],
                        func=AF.Relu if relu else AF.Identity,
                        bias=beT[:, o:o + 1], scale=gT[:, o:o + 1])
                    outs.append(t3)
                return outs

            def mlp_block(b):
                asl = slice(b * ABLK, (b + 1) * ABLK)
                hT = []
                for o in range(2):
                    ps = ps_x.tile([128, ABLK], FLOAT, tag='h')
                    nc.tensor.matmul(out=ps[:], lhsT=wg[:, o * 128:(o + 1) * 128],
                                     rhs=zt_sb[:, asl], start=True, stop=True)
                    sb = keep.tile([128, ABLK], BF16, tag=f'hT{o}_{b}')
                    nc.scalar.activation(out=sb[:], in_=ps[:], func=AF.Relu,
                                         bias=bgT[:, o:o + 1], scale=1.0)
                    hT.append(sb)

                sv1 = ln_block(hT, w1, 2, 4, b1T, g1T, be1T, ones1, relu=True)
                sv2 = ln_block(sv1, w2, 4, 2, b2T, g2T, be2T, ones2, relu=False)

                q_ps = ps_st.tile([1, ABLK], FLOAT, tag='q')
                for o in range(2):
                    av = ps_x.tile([128, ABLK], FLOAT, tag='av')
                    nc.tensor.matmul(out=av[:], lhsT=wa[:, o * 128:(o + 1) * 128],
                                     rhs=actT[:, asl], start=True, stop=True)
                    sav = mlp.tile([128, ABLK], BF16, tag='sav')
                    nc.vector.tensor_add(out=sav[:], in0=sv2[o][:], in1=av[:])
                    savr = mlp.tile([128, ABLK], BF16, tag='savr')
                    nc.scalar.activation(out=savr[:], in_=sav[:], func=AF.Relu,
                                         bias=zero_col[:, 0:1])
                    nc.tensor.matmul(out=q_ps[:], lhsT=wq[:, o:o + 1],
                                     rhs=savr[:], start=(o == 0), stop=(o == 1))
                q_sb = keep.tile([1, ABLK], FLOAT, tag=f'qsb{b}')
                nc.scalar.activation(out=q_sb[:], in_=q_ps[:],
                                     func=AF.Identity, bias=bq_sb[:, 0:1])
                nc.sync.dma_start(out=q_out[0:1, b * ABLK:(b + 1) * ABLK],
                                  in_=q_sb[:])

            # ------------- phase 1: aggregation -> zT [128, 1024] -------------
            for c in range(N_CHUNKS):
                gt = gathp.tile([128, CT, D_IN], BF16, tag='g')
                if c < 3:
                    # first use of each of the 3 buffers: ensure padding
                    # slots (trailing -1 idxs skip the write) hold finite
                    # data; later reuses inherit stale-but-finite rows.
                    nc.vector.memset(gt[:].rearrange('p k d -> p (k d)'), 0.0)
                for src, col0, n_idx, t0 in GATHERS:
                    x_src = xlo_t if src == 'lo' else xhi_t
                    nt = n_idx // 128
                    nc.gpsimd.dma_gather(
                        gt[:, t0:t0 + nt, :], x_src[:],
                        idx_sb[:, c * IDXW + col0: c * IDXW + col0 + n_idx // 16],
                        n_idx, n_idx, D_IN)
                st = smatp.tile([128, CT * AGG_CHUNK], BF16, tag='s')
                nc.sync.dma_start(out=st[:], in_=s_t[c])
                z_ps = ps_z.tile([D_IN, AGG_CHUNK], FLOAT, tag='z')
                for t in range(CT):
                    nc.tensor.matmul(
                        out=z_ps[:],
                        lhsT=gt[:, t, :],
                        rhs=st[:, t * AGG_CHUNK:(t + 1) * AGG_CHUNK],
                        start=(t == 0), stop=(t == CT - 1))
                nc.scalar.copy(
                    out=zt_sb[:, c * AGG_CHUNK:(c + 1) * AGG_CHUNK],
                    in_=z_ps[:])
                if c == N_CHUNKS // 2 - 1:
                    mlp_block(0)
            mlp_block(1)

    _split_multi_waits(nc)
    mybir.codegen_inst_isa_subclasses(nc)
    return nc


_NC_CACHE = None


def _get_program():
    global _NC_CACHE
    if _NC_CACHE is None:
        _NC_CACHE = _build_program()
    return _NC_CACHE


def _host_prep(x, edge_index, action, agent_idx, Wg, bg, W1, b1, g1, be1,
               W2, b2, g2, be2, Wa, ba, Wq, bq):
    """Graph preprocessing + per-core input maps."""
    src = np.asarray(edge_index[0], dtype=np.int64)
    dst = np.asarray(edge_index[1], dtype=np.int64)
    agent_idx = np.asarray(agent_idx, dtype=np.int64)

    cnt = np.bincount(dst, minlength=N_NODES)          # in-degree (no self)
    order = np.argsort(dst, kind='stable')
    src_s = src[order]
    indptr = np.zeros(N_NODES + 1, dtype=np.int64)
    np.cumsum(cnt, out=indptr[1:])
    deg = (cnt + 1).astype(np.float64)
    dinv = (1.0 / np.sqrt(deg)).astype(np.float32)

    # weights / biases shared by all cores
    x_bf = np.ascontiguousarray(np.asarray(x, np.float32)).astype(
        ml_dtypes.bfloat16)
    xlo = np.ascontiguousarray(x_bf[:SPLIT])
    xhi = np.ascontiguousarray(x_bf[SPLIT:])
    Wg_b = np.asarray(Wg, np.float32).astype(ml_dtypes.bfloat16)
    W1s = np.ascontiguousarray(
        np.asarray(W1, np.float32).reshape(2, 128, FC1)
        .transpose(1, 0, 2).reshape(128, 2 * FC1)).astype(ml_dtypes.bfloat16)
    W2s = np.ascontiguousarray(
        np.asarray(W2, np.float32).reshape(4, 128, FC2)
        .transpose(1, 0, 2).reshape(128, 4 * FC2)).astype(ml_dtypes.bfloat16)
    Wa_b = np.asarray(Wa, np.float32).astype(ml_dtypes.bfloat16)
    Wqs = np.ascontiguousarray(
        np.asarray(Wq, np.float32).reshape(2, 128).T).astype(ml_dtypes.bfloat16)
    biases = np.zeros((7, FC1), dtype=np.float32)
    biases[0, :D_HID] = bg
    biases[1] = b1
    biases[2] = g1
    biases[3] = be1
    biases[4, :FC2] = b2
    biases[5, :FC2] = g2
    biases[6, :FC2] = np.asarray(be2, np.float32) + np.asarray(ba, np.float32)
    biases[4, 256] = np.float32(np.asarray(bq).reshape(-1)[0])

    action = np.asarray(action, dtype=np.float32)

    in_maps = []
    for core in range(N_CORES):
        a0 = core * A_PER_CORE
        sidx = np.zeros((128, N_CHUNKS * IDXW), dtype=np.int16)
        smat = np.zeros((N_CHUNKS, 128, CT * AGG_CHUNK), dtype=ml_dtypes.bfloat16)
        for c in range(N_CHUNKS):
            lo_idx = np.zeros(N_LO, dtype=np.int64)
            hi_idx = np.zeros(N_HI, dtype=np.int64)
            sm = np.zeros((CT * 128, AGG_CHUNK), dtype=np.float32)
            v = agent_idx[a0 + c * AGG_CHUNK: a0 + (c + 1) * AGG_CHUNK]
            l = cnt[v]
            L = int(l.sum())
            ofs = np.repeat(
                indptr[v] - np.concatenate(([0], np.cumsum(l)[:-1])), l)
            epos = np.arange(L, dtype=np.int64) + ofs
            e_src = src_s[epos]
            e_acol = np.repeat(np.arange(AGG_CHUNK), l)
            e_norm = dinv[e_src] * dinv[np.repeat(v, l)]
            # self slots appended
            srcs = np.concatenate([e_src, v])
            acol = np.concatenate([e_acol, np.arange(AGG_CHUNK)])
            norm = np.concatenate([e_norm, dinv[v] * dinv[v]])
            is_lo = srcs < SPLIT
            for sel, base_t, tmax, idx_arr, rebase in (
                (is_lo, 0, T_LO, lo_idx, 0),
                (~is_lo, T_LO, T_HI, hi_idx, SPLIT),
            ):
                g_src = srcs[sel] - rebase
                g_acol = acol[sel]
                g_norm = norm[sel]
                n = g_src.shape[0]
                assert n <= tmax * 128, f'region slots {n} > {tmax * 128}'
                idx_arr[:n] = g_src
                # slot i -> tile base_t + i//128, partition i%128
                t_of = base_t + np.arange(n) // 128
                p_of = np.arange(n) % 128
                sm[t_of * 128 + p_of, g_acol] = g_norm
            both = np.concatenate([lo_idx, hi_idx]).astype(np.int16)
            sidx_c = both.reshape(IDXW, 16).T
            sidx[:, c * IDXW:(c + 1) * IDXW] = np.tile(sidx_c, (8, 1))
            # sm rows are (tile, partition) slots -> [128, CT*128] tile-major
            smat[c] = (sm.reshape(CT, 128, AGG_CHUNK).transpose(1, 0, 2)
                       .reshape(128, CT * AGG_CHUNK)).astype(ml_dtypes.bfloat16)
        in_maps.append({
            'xlo': xlo,
            'xhi': xhi,
            'sidx': sidx,
            'smat': smat,
            'actT': np.ascontiguousarray(
                action[a0:a0 + A_PER_CORE].T).astype(ml_dtypes.bfloat16),
            'Wg': Wg_b, 'W1s': W1s, 'W2s': W2s, 'Wa': Wa_b, 'Wqs': Wqs,
            'biases': biases,
        })
    return in_maps


_LAST_EXEC_NS = None


def kernel(trace=False, **inputs):
    global _LAST_EXEC_NS
    inputs = {k: np.asarray(v) for k, v in inputs.items()}
    in_maps = _host_prep(**inputs)
    nc = _get_program()
    res = run_bass_kernel_spmd(nc, in_maps, core_ids=list(range(N_CORES)),
                               trace=trace)
    _LAST_EXEC_NS = res.exec_time_ns
    q = np.concatenate([res.results[i]['q'][0] for i in range(N_CORES)])
    return q.reshape(N_AGENTS, 1).astype(np.float32)


# revision 17
# speedup vs baseline: 1.0562x; 1.0562x over previous
"""Trainium2 Bass kernel for nn_CriticNetwork (GCN critic head), 8 cores.

Math (reference): h = GCNConv(x, edge_index); sv = relu(h[agent_idx]);
sv = relu(LN(sv@W1+b1)); sv = LN(sv@W2+b2); q = relu(sv + action@Wa+ba) @ Wq + bq.

Exact algebraic restructurings (no approximation):
  * GCNConv is linear-then-propagate, so aggregate in the 128-d INPUT space
    and apply Wg after:  z[v] = sum_{e:dst=v} norm_e * x[src_e],
    h[v] = z[v] @ Wg + bg, with norm_e = dinv[src]*dinv[dst] and the self
    loop as one more edge (v->v, norm dinv[v]^2).  Only agent rows are ever
    used downstream, so only edges landing on agent nodes are aggregated
    (~121k of 800k).
  * Per-edge norm scaling + segment-sum fuse into one PE matmul per
    128-slot tile:  zT += G_t^T @ S_t  where G_t = gathered x rows
    [slot, feat] and S_t[slot, agent] = norm (0 off-target).  The output is
    directly transposed ([feat, agent]), which the whole MLP consumes.
  * ba folds into be2 (both add before the final relu).

vs. the first working version: the per-tile indirect DMAs (994ns fixed
SWDGE cost x 152) are replaced by 8 large dma_gather instructions (two per
2-chunk pair; x is split at row 32768 so gather indices fit int16), and all
matmul operands are bf16 (fp32 PSUM accumulation), halving PE stream time
and all HBM traffic.  MLP block b runs as soon as its 4 aggregation chunks
are done, overlapping with the remaining gathers.

Sharding: agents split 1024/core (data parallel); host work is graph
preprocessing only (CSR bucketing, degree/norm coefficients, index tables,
sparse S blocks) -- all feature-tensor FLOPs run on device.
"""

import numpy as np
import ml_dtypes

import concourse.bass as bass
import concourse.mybir as mybir
import concourse.tile as tile
from concourse import library_config
from concourse.bass_utils import run_bass_kernel_spmd

N_NODES = 50000
D_IN = 128
D_HID = 256
FC1 = 512
FC2 = 256
N_ACT = 64
N_AGENTS = 8192
LN_EPS = 1e-5

N_CORES = 8
A_PER_CORE = N_AGENTS // N_CORES        # 1024
ABLK = 512                              # agent block width for MLP
N_ABLK = A_PER_CORE // ABLK             # 2
AGG_CHUNK = 128                         # agents per aggregation chunk
N_CHUNKS = A_PER_CORE // AGG_CHUNK      # 8

SPLIT = 32768                           # x row split for int16 gather idx
T_LO = 12                               # slot tiles per chunk from x[:SPLIT]
T_HI = 7                                # slot tiles per chunk from x[SPLIT:]
CT = T_LO + T_HI                        # 20 tiles per chunk buffer
N_LO = T_LO * 128                       # 1664 lo idxs per chunk
N_HI = T_HI * 128                       # 896 hi idxs per chunk
IDXW = (N_LO + N_HI) // 16              # 160 idx columns per chunk
# dma_gather tops out at 1024 idxs per instruction (desc-ring capacity):
# split each chunk's lo stream into 1024 + 640.
GATHERS = (                             # (src, idx_col0, n_idx, tile0)
    ('lo', 0, 1024, 0),
    ('lo', 64, N_LO - 1024, 8),
    ('hi', N_LO // 16, N_HI, T_LO),
)

FLOAT = mybir.dt.float32
BF16 = mybir.dt.bfloat16
AF = mybir.ActivationFunctionType


def _split_multi_waits(nc, max_waits=1):
    """This container's walrus rejects >1 sync-wait per instruction; move
    extras onto same-engine NoOps inserted right before (equivalent)."""
    for func in nc.m.functions:
        for bb in func.blocks:
            out, changed = [], False
            for inst in bb.instructions:
                si = inst.sync_info
                if si is not None and len(si.on_wait) > max_waits:
                    waits = list(si.on_wait)
                    extra, keep = waits[:-max_waits], waits[-max_waits:]
                    for k in range(0, len(extra), max_waits):
                        nop = mybir.InstNoOp(
                            name=nc.get_next_instruction_name(),
                            engine=inst.engine, bass_nofuse=True,
                            sync_info=mybir.SyncInfo(
                                on_wait=list(extra[k:k + max_waits]),
                                on_update=[]))
                        nc.register_instruction(nop)
                        out.append(nop)
                        changed = True
                    si.on_wait.clear()
                    si.on_wait.extend(keep)
                    inst.sync_info = si
                out.append(inst)
            if changed:
                bb.instructions = out


def _build_program():
    nc = bass.Bass(target_bir_lowering=False)

    xlo_t = nc.declare_dram_parameter('xlo', [SPLIT, D_IN], BF16, isOutput=False)
    xhi_t = nc.declare_dram_parameter(
        'xhi', [N_NODES - SPLIT, D_IN], BF16, isOutput=False)
    idx_t = nc.declare_dram_parameter(
        'sidx', [128, N_CHUNKS * IDXW], mybir.dt.int16, isOutput=False)
    s_t = nc.declare_dram_parameter(
        'smat', [N_CHUNKS, 128, CT * AGG_CHUNK], BF16, isOutput=False)
    act_t = nc.declare_dram_parameter(
        'actT', [N_ACT, A_PER_CORE], BF16, isOutput=False)
    wg_t = nc.declare_dram_parameter('Wg', [D_IN, D_HID], BF16, isOutput=False)
    w1_t = nc.declare_dram_parameter('W1s', [128, 2 * FC1], BF16, isOutput=False)
    w2_t = nc.declare_dram_parameter('W2s', [128, 4 * FC2], BF16, isOutput=False)
    wa_t = nc.declare_dram_parameter('Wa', [N_ACT, FC2], BF16, isOutput=False)
    wq_t = nc.declare_dram_parameter('Wqs', [128, 2], BF16, isOutput=False)
    bias_t = nc.declare_dram_parameter('biases', [7, FC1], FLOAT, isOutput=False)
    q_out = nc.declare_dram_parameter('q', [1, A_PER_CORE], FLOAT, isOutput=True)

    with tile.TileContext(nc) as tc:
        with (
            tc.tile_pool(name='const', bufs=1) as constp,
            tc.tile_pool(name='gath', bufs=2) as gathp,
            tc.tile_pool(name='smatp', bufs=2) as smatp,
            tc.tile_pool(name='zt', bufs=1) as ztp,
            tc.tile_pool(name='ps_z', bufs=2, space='PSUM') as ps_z,
            tc.tile_pool(name='ps_y', bufs=2, space='PSUM') as ps_y,
            tc.tile_pool(name='ps_x', bufs=1, space='PSUM') as ps_x,
            tc.tile_pool(name='ps_st', bufs=1, space='PSUM') as ps_st,
            tc.tile_pool(name='mlp', bufs=2) as mlp,
            tc.tile_pool(name='mlp4', bufs=4) as mlp4,
            tc.tile_pool(name='keep', bufs=1) as keep,
        ):
            nc.gpsimd.load_library(library_config.mlp)

            # ---------------- constants ----------------
            idx_sb = constp.tile([128, N_CHUNKS * IDXW], mybir.dt.int16)
            nc.sync.dma_start(out=idx_sb[:], in_=idx_t[:])
            wg = constp.tile([D_IN, D_HID], BF16)
            nc.sync.dma_start(out=wg[:], in_=wg_t[:])
            w1 = constp.tile([128, 2 * FC1], BF16)
            nc.sync.dma_start(out=w1[:], in_=w1_t[:])
            w2 = constp.tile([128, 4 * FC2], BF16)
            nc.sync.dma_start(out=w2[:], in_=w2_t[:])
            wa = constp.tile([N_ACT, FC2], BF16)
            nc.sync.dma_start(out=wa[:], in_=wa_t[:])
            wq = constp.tile([128, 2], BF16)
            nc.sync.dma_start(out=wq[:], in_=wq_t[:])
            actT = constp.tile([N_ACT, A_PER_CORE], BF16)
            nc.sync.dma_start(out=actT[:], in_=act_t[:])
            ones1 = constp.tile([128, 128], BF16)
            nc.vector.memset(ones1[:], 1.0 / FC1)
            ones2 = constp.tile([128, 128], BF16)
            nc.vector.memset(ones2[:], 1.0 / FC2)
            zero_col = constp.tile([128, 1], FLOAT)
            nc.vector.memset(zero_col[:], 0.0)
            eps_col = constp.tile([128, 1], FLOAT)
            nc.vector.memset(eps_col[:], LN_EPS)

            def bias_col(row, n):
                t = constp.tile([128, n // 128], FLOAT, tag=f'bias{row}')
                nc.sync.dma_start(
                    out=t[:],
                    in_=bias_t[row, 0:n].rearrange('(k p) -> p k', p=128))
                return t

            bgT = bias_col(0, D_HID)
            b1T = bias_col(1, FC1)
            g1T = bias_col(2, FC1)
            be1T = bias_col(3, FC1)
            b2T = bias_col(4, FC2)
            g2T = bias_col(5, FC2)
            be2T = bias_col(6, FC2)   # includes +ba
            bq_sb = constp.tile([1, 1], FLOAT)
            nc.sync.dma_start(out=bq_sb[:], in_=bias_t[4:5, 256:257])

            zt_sb = ztp.tile([D_IN, A_PER_CORE], BF16)

            # ------------- phase 2: MLP (transposed activations) -------------
            def ln_block(in_tiles, w, nin, nout, bT, gT, beT, ones, relu):
                """y = w^T in + b; LN over the nout*128 feature axis
                (= partition axis across tiles); optional relu.  Returns
                SBUF tiles [128, ABLK] * nout."""
                y_sb = []
                for o in range(nout):
                    ps = ps_y.tile([128, ABLK], FLOAT, tag='y')
                    for k in range(nin):
                        nc.tensor.matmul(
                            out=ps[:],
                            lhsT=w[:, (k * nout + o) * 128:(k * nout + o + 1) * 128],
                            rhs=in_tiles[k][:],
                            start=(k == 0), stop=(k == nin - 1))
                    sb = mlp4.tile([128, ABLK], BF16, tag='ysb')
                    nc.scalar.activation(out=sb[:], in_=ps[:], func=AF.Identity,
                                         bias=bT[:, o:o + 1], scale=1.0)
                    y_sb.append(sb)
                mu = ps_st.tile([128, ABLK], FLOAT, tag='stat')
                for o in range(nout):
                    nc.tensor.matmul(out=mu[:], lhsT=ones[:], rhs=y_sb[o][:],
                                     start=(o == 0), stop=(o == nout - 1))
                d_sb, sq_sb = [], []
                for o in range(nout):
                    d = mlp4.tile([128, ABLK], BF16, tag='d')
                    nc.vector.tensor_sub(out=d[:], in0=y_sb[o][:], in1=mu[:])
                    d_sb.append(d)
                    s = mlp4.tile([128, ABLK], BF16, tag='sq')
                    nc.scalar.activation(out=s[:], in_=d[:], func=AF.Square,
                                         bias=zero_col[:, 0:1])
                    sq_sb.append(s)
                var = ps_st.tile([128, ABLK], FLOAT, tag='stat')
                for o in range(nout):
                    nc.tensor.matmul(out=var[:], lhsT=ones[:], rhs=sq_sb[o][:],
                                     start=(o == 0), stop=(o == nout - 1))
                lg = mlp.tile([128, ABLK], FLOAT, tag='lg')
                nc.scalar.activation(out=lg[:], in_=var[:], func=AF.Ln,
                                     bias=eps_col[:, 0:1])
                r = mlp.tile([128, ABLK], BF16, tag='r')
                nc.scalar.activation(out=r[:], in_=lg[:], func=AF.Exp,
                                     bias=zero_col[:, 0:1], scale=-0.5)
                outs = []
                for o in range(nout):
                    t1 = mlp.tile([128, ABLK], BF16, tag='t1')
                    nc.vector.tensor_mul(out=t1[:], in0=d_sb[o][:], in1=r[:])
                    t3 = mlp4.tile([128, ABLK], BF16, tag='t3')
                    nc.scalar.activation(
                        out=t3[:], in_=t1[:],
                        func=AF.Relu if relu else AF.Identity,
                        bias=beT[:, o:o + 1], scale=gT[:, o:o + 1])
                    outs.append(t3)
                return outs

            def mlp_block(b):
                asl = slice(b * ABLK, (b + 1) * ABLK)
                hT = []
                for o in range(2):
                    ps = ps_x.tile([128, ABLK], FLOAT, tag='h')
                    nc.tensor.matmul(out=ps[:], lhsT=wg[:, o * 128:(o + 1) * 128],
                                     rhs=zt_sb[:, asl], start=True, stop=True)
                    sb = keep.tile([128, ABLK], BF16, tag=f'hT{o}_{b}')
                    nc.scalar.activation(out=sb[:], in_=ps[:], func=AF.Relu,
                                         bias=bgT[:, o:o + 1], scale=1.0)
                    hT.append(sb)

                sv1 = ln_block(hT, w1, 2, 4, b1T, g1T, be1T, ones1, relu=True)
                sv2 = ln_block(sv1, w2, 4, 2, b2T, g2T, be2T, ones2, relu=False)

                q_ps = ps_st.tile([1, ABLK], FLOAT, tag='q')
                for o in range(2):
                    av = ps_x.tile([128, ABLK], FLOAT, tag='av')
                    nc.tensor.matmul(out=av[:], lhsT=wa[:, o * 128:(o + 1) * 128],
                                     rhs=actT[:, asl], start=True, stop=True)
                    sav = mlp.tile([128, ABLK], BF16, tag='sav')
                    nc.vector.tensor_add(out=sav[:], in0=sv2[o][:], in1=av[:])
                    savr = mlp.tile([128, ABLK], BF16, tag='savr')
                    nc.scalar.activation(out=savr[:], in_=sav[:], func=AF.Relu,
                                         bias=zero_col[:, 0:1])
                    nc.tensor.matmul(out=q_ps[:], lhsT=wq[:, o:o + 1],
                                     rhs=savr[:], start=(o == 0), stop=(o == 1))
                q_sb = keep.tile([1, ABLK], FLOAT, tag=f'qsb{b}')
                nc.scalar.activation(out=q_sb[:], in_=q_ps[:],
                                     func=AF.Identity, bias=bq_sb[:, 0:1])
                nc.sync.dma_start(out=q_out[0:1, b * ABLK:(b + 1) * ABLK],
                                  in_=q_sb[:])

            # ------------- phase 1: aggregation -> zT [128, 1024] -------------
            for c in range(N_CHUNKS):
                gt = gathp.tile([128, CT, D_IN], BF16, tag='g')
                if c < 2:
                    # first use of each of the 3 buffers: ensure padding
                    # slots (trailing -1 idxs skip the write) hold finite
                    # data; later reuses inherit stale-but-finite rows.
                    nc.vector.memset(gt[:].rearrange('p k d -> p (k d)'), 0.0)
                for src, col0, n_idx, t0 in GATHERS:
                    x_src = xlo_t if src == 'lo' else xhi_t
                    nt = n_idx // 128
                    nc.gpsimd.dma_gather(
                        gt[:, t0:t0 + nt, :], x_src[:],
                        idx_sb[:, c * IDXW + col0: c * IDXW + col0 + n_idx // 16],
                        n_idx, n_idx, D_IN)
                st = smatp.tile([128, CT * AGG_CHUNK], BF16, tag='s')
                nc.sync.dma_start(out=st[:], in_=s_t[c])
                z_ps = ps_z.tile([D_IN, AGG_CHUNK], FLOAT, tag='z')
                for t in range(CT):
                    nc.tensor.matmul(
                        out=z_ps[:],
                        lhsT=gt[:, t, :],
                        rhs=st[:, t * AGG_CHUNK:(t + 1) * AGG_CHUNK],
                        start=(t == 0), stop=(t == CT - 1))
                nc.scalar.copy(
                    out=zt_sb[:, c * AGG_CHUNK:(c + 1) * AGG_CHUNK],
                    in_=z_ps[:])
                if c == N_CHUNKS // 2 - 1:
                    mlp_block(0)
            mlp_block(1)

    _split_multi_waits(nc)
    mybir.codegen_inst_isa_subclasses(nc)
    return nc


_NC_CACHE = None


def _get_program():
    global _NC_CACHE
    if _NC_CACHE is None:
        _NC_CACHE = _build_program()
    return _NC_CACHE


def _host_prep(x, edge_index, action, agent_idx, Wg, bg, W1, b1, g1, be1,
               W2, b2, g2, be2, Wa, ba, Wq, bq):
    """Graph preprocessing + per-core input maps."""
    src = np.asarray(edge_index[0], dtype=np.int64)
    dst = np.asarray(edge_index[1], dtype=np.int64)
    agent_idx = np.asarray(agent_idx, dtype=np.int64)

    cnt = np.bincount(dst, minlength=N_NODES)          # in-degree (no self)
    order = np.argsort(dst, kind='stable')
    src_s = src[order]
    indptr = np.zeros(N_NODES + 1, dtype=np.int64)
    np.cumsum(cnt, out=indptr[1:])
    deg = (cnt + 1).astype(np.float64)
    dinv = (1.0 / np.sqrt(deg)).astype(np.float32)

    # weights / biases shared by all cores
    x_bf = np.ascontiguousarray(np.asarray(x, np.float32)).astype(
        ml_dtypes.bfloat16)
    xlo = np.ascontiguousarray(x_bf[:SPLIT])
    xhi = np.ascontiguousarray(x_bf[SPLIT:])
    Wg_b = np.asarray(Wg, np.float32).astype(ml_dtypes.bfloat16)
    W1s = np.ascontiguousarray(
        np.asarray(W1, np.float32).reshape(2, 128, FC1)
        .transpose(1, 0, 2).reshape(128, 2 * FC1)).astype(ml_dtypes.bfloat16)
    W2s = np.ascontiguousarray(
        np.asarray(W2, np.float32).reshape(4, 128, FC2)
        .transpose(1, 0, 2).reshape(128, 4 * FC2)).astype(ml_dtypes.bfloat16)
    Wa_b = np.asarray(Wa, np.float32).astype(ml_dtypes.bfloat16)
    Wqs = np.ascontiguousarray(
        np.asarray(Wq, np.float32).reshape(2, 128).T).astype(ml_dtypes.bfloat16)
    biases = np.zeros((7, FC1), dtype=np.float32)
    biases[0, :D_HID] = bg
    biases[1] = b1
    biases[2] = g1
    biases[3] = be1
    biases[4, :FC2] = b2
    biases[5, :FC2] = g2
    biases[6, :FC2] = np.asarray(be2, np.float32) + np.asarray(ba, np.float32)
    biases[4, 256] = np.float32(np.asarray(bq).reshape(-1)[0])

    action = np.asarray(action, dtype=np.float32)

    in_maps = []
    for core in range(N_CORES):
        a0 = core * A_PER_CORE
        sidx = np.zeros((128, N_CHUNKS * IDXW), dtype=np.int16)
        smat = np.zeros((N_CHUNKS, 128, CT * AGG_CHUNK), dtype=ml_dtypes.bfloat16)
        for c in range(N_CHUNKS):
            lo_idx = np.zeros(N_LO, dtype=np.int64)
            hi_idx = np.zeros(N_HI, dtype=np.int64)
            sm = np.zeros((CT * 128, AGG_CHUNK), dtype=np.float32)
            v = agent_idx[a0 + c * AGG_CHUNK: a0 + (c + 1) * AGG_CHUNK]
            l = cnt[v]
            L = int(l.sum())
            ofs = np.repeat(
                indptr[v] - np.concatenate(([0], np.cumsum(l)[:-1])), l)
            epos = np.arange(L, dtype=np.int64) + ofs
            e_src = src_s[epos]
            e_acol = np.repeat(np.arange(AGG_CHUNK), l)
            e_norm = dinv[e_src] * dinv[np.repeat(v, l)]
            # self slots appended
            srcs = np.concatenate([e_src, v])
            acol = np.concatenate([e_acol, np.arange(AGG_CHUNK)])
            norm = np.concatenate([e_norm, dinv[v] * dinv[v]])
            is_lo = srcs < SPLIT
            for sel, base_t, tmax, idx_arr, rebase in (
                (is_lo, 0, T_LO, lo_idx, 0),
                (~is_lo, T_LO, T_HI, hi_idx, SPLIT),
            ):
                g_src = srcs[sel] - rebase
                g_acol = acol[sel]
                g_norm = norm[sel]
                n = g_src.shape[0]
                assert n <= tmax * 128, f'region slots {n} > {tmax * 128}'
                idx_arr[:n] = g_src
                # slot i -> tile base_t + i//128, partition i%128
                t_of = base_t + np.arange(n) // 128
                p_of = np.arange(n) % 128
                sm[t_of * 128 + p_of, g_acol] = g_norm
            both = np.concatenate([lo_idx, hi_idx]).astype(np.int16)
            sidx_c = both.reshape(IDXW, 16).T
            sidx[:, c * IDXW:(c + 1) * IDXW] = np.tile(sidx_c, (8, 1))
            # sm rows are (tile, partition) slots -> [128, CT*128] tile-major
            smat[c] = (sm.reshape(CT, 128, AGG_CHUNK).transpose(1, 0, 2)
                       .reshape(128, CT * AGG_CHUNK)).astype(ml_dtypes.bfloat16)
        in_maps.append({
            'xlo': xlo,
            'xhi': xhi,
            'sidx': sidx,
            'smat': smat,
            'actT': np.ascontiguousarray(
                action[a0:a0 + A_PER_CORE].T).astype(ml_dtypes.bfloat16),
            'Wg': Wg_b, 'W1s': W1s, 'W2s': W2s, 'Wa': Wa_b, 'Wqs': Wqs,
            'biases': biases,
        })
    return in_maps


_LAST_EXEC_NS = None


def kernel(trace=False, **inputs):
    global _LAST_EXEC_NS
    inputs = {k: np.asarray(v) for k, v in inputs.items()}
    in_maps = _host_prep(**inputs)
    nc = _get_program()
    res = run_bass_kernel_spmd(nc, in_maps, core_ids=list(range(N_CORES)),
                               trace=trace)
    _LAST_EXEC_NS = res.exec_time_ns
    q = np.concatenate([res.results[i]['q'][0] for i in range(N_CORES)])
    return q.reshape(N_AGENTS, 1).astype(np.float32)
